# revision 11
# baseline (speedup 1.0000x reference)
"""Trainium2 Bass kernel for nn_DLGCNLayer (2-hop bipartite GNN layer).

Computes, for a bipartite user-item graph given as COO edges:
    hV = segsum_v(val * ufea[u])            # [n_items, D]   (VU @ ufea)
    hU = segsum_u(val * vfea[v])            # [n_users, D]   (UV @ vfea)
    gU = segsum_u(val * hV[v])              # [n_users, D]   (UV @ hV)
    gV = segsum_v(val * hU[u])              # [n_items, D]   (VU @ hU)
    User = relu(cat(gU, ufea) @ user_W.T + user_b)
    Item = relu(cat(gV, vfea) @ item_W.T + item_b)

Sharding: output nodes are split evenly across 8 cores; edges are
partitioned by destination node (1D graph partition), once per direction.
Stage-1 partial tensors are exchanged with chunked AllGathers that overlap
compute.  Each segment sum runs as PE matmuls over 128-edge blocks with
one-hot selection matrices built on the DVE (one tensor_scalar per block);
source rows are fetched in bulk with GPSIMD dma_gather (int16 indices, so
edge streams are segmented by 32768-row source ranges).
"""

from contextlib import ExitStack

import ml_dtypes
import numpy as np

import concourse.bass as bass
import concourse.mybir as mybir
import concourse.tile as tile
from concourse import bacc
from concourse.bass_utils import run_bass_kernel_spmd

P = 128
C = 8
BF16 = mybir.dt.bfloat16
F32 = mybir.dt.float32
I16 = mybir.dt.int16
NPBF16 = ml_dtypes.bfloat16

RANGE = 32768      # dma_gather int16 index reach (rows)
TILE_BATCH = 6     # dest tiles accumulating in PSUM concurrently (6 banks)
G_BLOCKS = 32      # max 128-edge blocks per dma_gather group

LAST_RESULTS = None  # BassKernelResults of the last kernel() call


# --------------------------------------------------------------------------
# Host-side preprocessing
# --------------------------------------------------------------------------

class StageLayout:
    """Shared-across-cores stream layout for one (ordering, stage):
    blocks of 128 edges, organized [tile-batch][source-range][tile], with
    per-(tile,range) budgets = max over cores (rounded to blocks)."""

    def __init__(self, NT, table_rows):
        self.NT = NT
        self.table_rows = table_rows
        self.R = (table_rows + RANGE - 1) // RANGE

    def build(self, core, t, dl, src, val):
        NT, R = self.NT, self.R
        r = src // RANGE
        key = (core * NT + t) * R + r
        counts = np.bincount(key, minlength=C * NT * R).reshape(C, NT, R)
        nb3 = (counts.max(axis=0) + P - 1) // P        # [NT, R] blocks
        empty = nb3.sum(axis=1) == 0
        nb3[empty, 0] = 1                              # keep every tile alive

        # stream schedule: [batch][range][tile] -> blocks
        sched = []        # block -> tile
        brange = []       # block -> range id
        offs = np.zeros((NT, R), np.int64)             # (t,r) -> first block
        for b0 in range(0, NT, TILE_BATCH):
            batch = range(b0, min(b0 + TILE_BATCH, NT))
            for rr in range(R):
                for t_ in batch:
                    n = int(nb3[t_, rr])
                    if n == 0:
                        continue
                    offs[t_, rr] = len(sched)
                    sched += [t_] * n
                    brange += [rr] * n
        NB = len(sched)
        sched = np.asarray(sched, np.int64)
        brange = np.asarray(brange, np.int64)

        first = np.full(NT, -1, np.int64)
        last = np.full(NT, -1, np.int64)
        for b, t_ in enumerate(sched):
            if first[t_] < 0:
                first[t_] = b
            last[t_] = b

        # gather groups: cut on range change or G_BLOCKS
        groups = []  # (start_block, n_blocks, row0, row1)
        gs = 0
        for b in range(1, NB + 1):
            if b == NB or brange[b] != brange[gs] or b - gs == G_BLOCKS:
                rr = int(brange[gs])
                groups.append(
                    (gs, b - gs, rr * RANGE, min((rr + 1) * RANGE, self.table_rows))
                )
                gs = b
        block_group = np.zeros(NB, np.int64)
        for gi, (b0, n, _, _) in enumerate(groups):
            block_group[b0 : b0 + n] = gi

        # place edges: position within (core, t, r) group
        order = np.lexsort((r, t, core))
        coreo, to, ro = core[order], t[order], r[order]
        dlo, srco, valo = dl[order], src[order], val[order]
        keyo = (coreo * NT + to) * R + ro
        grp_start = np.concatenate([[0], np.cumsum(counts.reshape(-1))])[:-1]
        pos = np.arange(len(keyo)) - grp_start[keyo]
        eidx = offs[to, ro] * P + pos                  # slot in padded stream

        L = NB * P
        dst_p = np.zeros((C, L), np.float32)
        val_p = np.zeros((C, L), np.float32)
        rel_p = np.zeros((C, L), np.int64)
        dst_p[coreo, eidx] = dlo
        val_p[coreo, eidx] = valo
        rel_p[coreo, eidx] = srco - ro * RANGE
        assert rel_p.max() < RANGE and rel_p.min() >= 0
        # padded slots: rel 0 is a valid row of every range, val 0 kills
        # their contribution.

        self.NB = NB
        self.sched = sched
        self.groups = groups
        self.block_group = block_group
        self.first = first
        self.last = last
        # SBUF layouts: edge e -> (partition e%128, col e//128)
        T = lambda a, dt: np.ascontiguousarray(
            a.reshape(C, NB, P).transpose(0, 2, 1)
        ).astype(dt)
        self.dst = T(dst_p, np.float32)
        self.val = T(val_p, np.float32)
        # int16 indices wrapped into 16 partitions, replicated to 128:
        # index i -> (partition i%16, col i//16)
        w = rel_p.reshape(C, NB * 8, 16).transpose(0, 2, 1).astype(np.int16)
        self.idx16 = np.ascontiguousarray(np.tile(w, (1, 8, 1)))  # [C,128,NB*8]
        return self


def _split_by_dest(dst, n_shard):
    core = dst // n_shard
    loc = dst % n_shard
    return core, loc // P, (loc % P).astype(np.float32)


def _ag_splits(NT, n_shard, n_chunks):
    bounds = [0]
    step = max(1, NT // n_chunks)
    for k in range(1, n_chunks):
        rows = min(step * k * P, n_shard)
        if rows > bounds[-1] and rows < n_shard:
            bounds.append(rows)
    bounds.append(n_shard)
    return bounds


def _ag_remap(n_nodes, n_shard, splits):
    """node id -> row in the chunk-major AllGather output tensor."""
    node = np.arange(n_nodes)
    c = node // n_shard
    loc = node % n_shard
    out = np.zeros(n_nodes, np.int64)
    base = 0
    for k in range(len(splits) - 1):
        s0, s1 = splits[k], splits[k + 1]
        rows = s1 - s0
        m = (loc >= s0) & (loc < s1)
        out[m] = base + c[m] * rows + (loc[m] - s0)
        base += C * rows
    return out


# --------------------------------------------------------------------------
# Kernel builder
# --------------------------------------------------------------------------

def _build(cfg):
    n_users = cfg["n_users"]
    n_items = cfg["n_items"]
    U_SH = n_users // C
    V_SH = n_items // C
    splits_u = cfg["splits_u"]
    splits_v = cfg["splits_v"]
    layouts = cfg["layouts"]  # (ordering+stage) -> StageLayout

    nc = bacc.Bacc(
        "TRN2",
        target_bir_lowering=False,
        debug=False,
        enable_asserts=False,
        num_devices=C,
    )

    ufeaB = nc.dram_tensor("ufeab", [n_users, P], BF16, kind="ExternalInput")
    vfeaB = nc.dram_tensor("vfeab", [n_items, P], BF16, kind="ExternalInput")
    # per-core transposed feature slices (for the dense stage)
    ufeaT = nc.dram_tensor("ufeat", [P, U_SH], BF16, kind="ExternalInput")
    vfeaT = nc.dram_tensor("vfeat", [P, V_SH], BF16, kind="ExternalInput")

    io = {}
    for os_ in ("u1", "u2", "v1", "v2"):
        NB = layouts[os_].NB
        io[os_ + "_dst"] = nc.dram_tensor(os_ + "_dst", [P, NB], F32, kind="ExternalInput")
        io[os_ + "_val"] = nc.dram_tensor(os_ + "_val", [P, NB], F32, kind="ExternalInput")
        io[os_ + "_idx"] = nc.dram_tensor(os_ + "_idx", [P, NB * 8], I16, kind="ExternalInput")

    wu0 = nc.dram_tensor("wu0", [P, P], BF16, kind="ExternalInput")
    wu1 = nc.dram_tensor("wu1", [P, P], BF16, kind="ExternalInput")
    wi0 = nc.dram_tensor("wi0", [P, P], BF16, kind="ExternalInput")
    wi1 = nc.dram_tensor("wi1", [P, P], BF16, kind="ExternalInput")
    bu = nc.dram_tensor("bu", [1, P], BF16, kind="ExternalInput")
    bi = nc.dram_tensor("bi", [1, P], BF16, kind="ExternalInput")

    iota_np = np.tile(np.arange(P, dtype=np.float32), (P, 1)).astype(NPBF16)
    iota_d = nc.inline_tensor(iota_np, "iotab")
    ones_d = nc.inline_tensor(np.ones((1, P), NPBF16), "onesb")

    user_out = nc.dram_tensor("user_out", [U_SH, P], F32, kind="ExternalOutput")
    item_out = nc.dram_tensor("item_out", [V_SH, P], F32, kind="ExternalOutput")

    with tile.TileContext(nc) as tc, ExitStack() as ctx:
        const = ctx.enter_context(tc.tile_pool(name="const", bufs=1))
        meta = ctx.enter_context(tc.tile_pool(name="meta", bufs=1))
        dram = ctx.enter_context(tc.tile_pool(name="dram", bufs=1, space="DRAM"))
        gat = ctx.enter_context(tc.tile_pool(name="gat", bufs=3))
        idxp = ctx.enter_context(tc.tile_pool(name="idxp", bufs=4))
        selp = ctx.enter_context(tc.tile_pool(name="sel", bufs=8))
        psA = ctx.enter_context(tc.tile_pool(name="psA", bufs=TILE_BATCH, space="PSUM"))
        psD = ctx.enter_context(tc.tile_pool(name="psD", bufs=2, space="PSUM"))
        s1o = ctx.enter_context(tc.tile_pool(name="s1o", bufs=3))
        hoTp = ctx.enter_context(tc.tile_pool(name="hoT", bufs=2))
        feaTp = ctx.enter_context(tc.tile_pool(name="feaT", bufs=2))
        dout = ctx.enter_context(tc.tile_pool(name="dout", bufs=3))

        iota_sb = const.tile([P, P], BF16, tag="iota", name="iota")
        nc.sync.dma_start(out=iota_sb[:], in_=iota_d[:, :])
        ones_sb = const.tile([1, P], BF16, tag="ones", name="ones")
        nc.sync.dma_start(out=ones_sb[:], in_=ones_d[:, :])
        w_sb = {}
        for nm, h in (("wu0", wu0), ("wu1", wu1), ("wi0", wi0), ("wi1", wi1)):
            w_sb[nm] = const.tile([P, P], BF16, tag=nm, name=nm)
            nc.sync.dma_start(out=w_sb[nm][:], in_=h[:, :])
        b_sb = {}
        for nm, h in (("bu", bu), ("bi", bi)):
            b_sb[nm] = const.tile([1, P], BF16, tag=nm, name=nm)
            nc.sync.dma_start(out=b_sb[nm][:], in_=h[:, :])

        m_sb = {}
        for os_ in ("u1", "u2", "v1", "v2"):
            for f in ("dst", "val"):
                nm = os_ + "_" + f
                h = io[nm]
                m_sb[nm] = meta.tile(list(h.shape), F32, tag=nm, name=nm)
                nc.sync.dma_start(out=m_sb[nm][:], in_=h[:, :])

        # AllGather buffers: the collective writes one Shared tensor per
        # chunk (single-writer rule); chunks are DMA-merged into one Local
        # gather table.
        ag_in_hV = dram.tile([V_SH, P], BF16, tag="aginV", name="ag_in_hV")
        ag_out_hV = dram.tile([C * V_SH, P], BF16, tag="agoV", name="ag_out_hV")
        ag_in_hU = dram.tile([U_SH, P], BF16, tag="aginU", name="ag_in_hU")
        ag_out_hU = dram.tile([C * U_SH, P], BF16, tag="agoU", name="ag_out_hU")
        ag_sh = {}
        for o, splits in (("v", splits_v), ("u", splits_u)):
            for k in range(len(splits) - 1):
                rows = splits[k + 1] - splits[k]
                ag_sh[(o, k)] = dram.tile(
                    [C * rows, P], BF16, tag=f"agsh{o}{k}",
                    name=f"agsh{o}{k}", addr_space="Shared",
                )

        rg = [list(range(C))]

        def ag_chunk(o, ag_in, ag_out, splits, k, base_rows):
            s0, s1 = splits[k], splits[k + 1]
            rows = s1 - s0
            sh = ag_sh[(o, k)]
            nc.gpsimd.collective_compute(
                "AllGather",
                mybir.AluOpType.bypass,
                replica_groups=rg,
                ins=[ag_in[s0:s1, :].opt()],
                outs=[sh[:, :].opt()],
            )
            nc.sync.dma_start(
                out=ag_out[base_rows : base_rows + C * rows, :], in_=sh[:, :]
            )
            return base_rows + C * rows

        def spmm(os_, table_ap, orient, on_tile_done):
            """Emit the full block stream for one (ordering, stage)."""
            L = layouts[os_]
            dst_sb = m_sb[os_ + "_dst"]
            val_sb = m_sb[os_ + "_val"]
            idx_dram = io[os_ + "_idx"]
            cur_g = -1
            gt = None
            gb0 = 0
            ps_live = {}
            for b in range(L.NB):
                g = int(L.block_group[b])
                if g != cur_g:
                    b0, nbg, row0, row1 = L.groups[g]
                    S = nbg * 8
                    it = idxp.tile([P, S], I16, tag="idx", name="idx")
                    nc.sync.dma_start(
                        out=it[:], in_=idx_dram[:, b0 * 8 : b0 * 8 + S]
                    )
                    gt = gat.tile([P, nbg, P], BF16, tag="g", name="g")
                    nc.gpsimd.dma_gather(
                        gt[:], table_ap[row0:row1, :], it[:],
                        nbg * P, nbg * P, P, single_packet=False,
                    )
                    cur_g = g
                    gb0 = b0
                t = int(L.sched[b])
                if t not in ps_live:
                    ps_live[t] = psA.tile([P, P], F32, tag="acc", name="acc")
                ps = ps_live[t]
                s = selp.tile([P, P], BF16, tag="sel", name="sel")
                nc.vector.tensor_scalar(
                    out=s[:],
                    in0=iota_sb[:],
                    scalar1=dst_sb[:, b : b + 1],
                    scalar2=val_sb[:, b : b + 1],
                    op0=mybir.AluOpType.is_equal,
                    op1=mybir.AluOpType.mult,
                )
                start = b == L.first[t]
                stop = b == L.last[t]
                blk = gt[:, b - gb0, :]
                if orient == "s1":
                    nc.tensor.matmul(ps[:], lhsT=s[:], rhs=blk, start=start, stop=stop)
                else:
                    nc.tensor.matmul(ps[:], lhsT=blk, rhs=s[:], start=start, stop=stop)
                if stop:
                    on_tile_done(t, ps)
                    del ps_live[t]

        def s1_done(ag_in, n_rows, o, ag_out, splits, last_tiles):
            state = {"base": 0}

            def done(t, ps):
                o1 = s1o.tile([P, P], BF16, tag="o1", name="o1")
                nc.vector.tensor_copy(out=o1[:], in_=ps[:])
                r0 = t * P
                r1 = min(r0 + P, n_rows)
                nc.sync.dma_start(out=ag_in[r0:r1, :], in_=o1[: r1 - r0, :])
                if t in last_tiles:
                    k = last_tiles[t]
                    state["base"] = ag_chunk(o, ag_in, ag_out, splits, k, state["base"])

            return done

        def s2_done(n_rows, feaT_h, w0, w1, bias, out_h):
            def done(t, ps):
                hoT = hoTp.tile([P, P], BF16, tag="hoT", name="hoT")
                nc.vector.tensor_copy(out=hoT[:], in_=ps[:])
                r0 = t * P
                r1 = min(r0 + P, n_rows)
                nrow = r1 - r0
                fT = feaTp.tile([P, P], BF16, tag="fT", name="fT")
                if nrow < P:
                    nc.vector.memset(fT[:], 0.0)
                nc.sync.dma_start(out=fT[:, :nrow], in_=feaT_h[:, r0:r1])
                pd = psD.tile([P, P], F32, tag="d", name="d")
                nc.tensor.matmul(pd[:], lhsT=hoT[:], rhs=w0[:], start=True, stop=False)
                nc.tensor.matmul(pd[:], lhsT=fT[:], rhs=w1[:], start=False, stop=False)
                nc.tensor.matmul(pd[:], lhsT=ones_sb[:], rhs=bias[:], start=False, stop=True)
                ob = dout.tile([P, P], F32, tag="ob", name="ob")
                nc.scalar.activation(
                    out=ob[:], in_=pd[:], func=mybir.ActivationFunctionType.Relu
                )
                nc.sync.dma_start(out=out_h[r0:r1, :], in_=ob[:nrow, :])

            return done

        def chunk_last_tiles(splits):
            return {
                (splits[k + 1] + P - 1) // P - 1: k for k in range(len(splits) - 1)
            }

        # ---- emission order ----------------------------------------------
        spmm(
            "v1", ufeaB.ap(), "s1",
            s1_done(ag_in_hV, V_SH, "v", ag_out_hV, splits_v,
                    chunk_last_tiles(splits_v)),
        )
        spmm(
            "u1", vfeaB.ap(), "s1",
            s1_done(ag_in_hU, U_SH, "u", ag_out_hU, splits_u,
                    chunk_last_tiles(splits_u)),
        )
        spmm(
            "u2", ag_out_hV[:, :], "s2",
            s2_done(U_SH, ufeaT, w_sb["wu0"], w_sb["wu1"], b_sb["bu"], user_out),
        )
        spmm(
            "v2", ag_out_hU[:, :], "s2",
            s2_done(V_SH, vfeaT, w_sb["wi0"], w_sb["wi1"], b_sb["bi"], item_out),
        )

    nc.compile()
    return nc


# --------------------------------------------------------------------------
# Entry point
# --------------------------------------------------------------------------

def prepare(
    ufea, vfea, edge_u, edge_v, edge_val, user_W, user_b, item_W, item_b,
    n_ag_chunks=2,
):
    ufea = np.asarray(ufea, np.float32)
    vfea = np.asarray(vfea, np.float32)
    edge_u = np.asarray(edge_u).astype(np.int64)
    edge_v = np.asarray(edge_v).astype(np.int64)
    edge_val = np.asarray(edge_val, np.float32)
    user_W = np.asarray(user_W, np.float32)
    user_b = np.asarray(user_b, np.float32)
    item_W = np.asarray(item_W, np.float32)
    item_b = np.asarray(item_b, np.float32)

    n_users, D = ufea.shape
    n_items = vfea.shape[0]
    assert D == P and n_users % C == 0 and n_items % C == 0
    U_SH = n_users // C
    V_SH = n_items // C
    NT_u = (U_SH + P - 1) // P
    NT_v = (V_SH + P - 1) // P

    splits_u = _ag_splits(NT_u, U_SH, n_ag_chunks)
    splits_v = _ag_splits(NT_v, V_SH, n_ag_chunks)
    remap_hU = _ag_remap(n_users, U_SH, splits_u)
    remap_hV = _ag_remap(n_items, V_SH, splits_v)

    # u-ordering: dest = users, sources = items
    cu, tu, dlu = _split_by_dest(edge_u, U_SH)
    # v-ordering: dest = items, sources = users
    cv, tv, dlv = _split_by_dest(edge_v, V_SH)

    layouts = {
        "u1": StageLayout(NT_u, n_items).build(cu, tu, dlu, edge_v, edge_val),
        "u2": StageLayout(NT_u, C * V_SH).build(cu, tu, dlu, remap_hV[edge_v], edge_val),
        "v1": StageLayout(NT_v, n_users).build(cv, tv, dlv, edge_u, edge_val),
        "v2": StageLayout(NT_v, C * U_SH).build(cv, tv, dlv, remap_hU[edge_u], edge_val),
    }

    cfg = dict(
        n_users=n_users,
        n_items=n_items,
        splits_u=splits_u,
        splits_v=splits_v,
        layouts=layouts,
    )
    nc = _build(cfg)

    ufea_b = ufea.astype(NPBF16)
    vfea_b = vfea.astype(NPBF16)
    uT = np.ascontiguousarray(ufea_b.T)
    vT = np.ascontiguousarray(vfea_b.T)
    wu = user_W.T.astype(NPBF16)
    wi = item_W.T.astype(NPBF16)

    in_maps = []
    for c in range(C):
        m = {
            "ufeab": ufea_b,
            "vfeab": vfea_b,
            "ufeat": np.ascontiguousarray(uT[:, c * U_SH : (c + 1) * U_SH]),
            "vfeat": np.ascontiguousarray(vT[:, c * V_SH : (c + 1) * V_SH]),
            "wu0": np.ascontiguousarray(wu[:P]),
            "wu1": np.ascontiguousarray(wu[P:]),
            "wi0": np.ascontiguousarray(wi[:P]),
            "wi1": np.ascontiguousarray(wi[P:]),
            "bu": user_b.astype(NPBF16)[None, :],
            "bi": item_b.astype(NPBF16)[None, :],
        }
        for os_ in ("u1", "u2", "v1", "v2"):
            L = layouts[os_]
            m[os_ + "_dst"] = L.dst[c]
            m[os_ + "_val"] = L.val[c]
            m[os_ + "_idx"] = L.idx16[c]
        in_maps.append(m)

    return nc, in_maps


def kernel(
    ufea, vfea, edge_u, edge_v, edge_val, user_W, user_b, item_W, item_b,
    n_ag_chunks=2,
):
    global LAST_RESULTS
    nc, in_maps = prepare(
        ufea, vfea, edge_u, edge_v, edge_val,
        user_W, user_b, item_W, item_b, n_ag_chunks=n_ag_chunks,
    )
    res = run_bass_kernel_spmd(nc, in_maps, core_ids=list(range(C)))
    LAST_RESULTS = res

    User = np.concatenate([res.results[c]["user_out"] for c in range(C)], axis=0)
    Item = np.concatenate([res.results[c]["item_out"] for c in range(C)], axis=0)
    return User, Item
